# revision 30
# baseline (speedup 1.0000x reference)
"""Multi-head causal attention (B=4, S=2048, H=1024, 16 heads) on 8 TRN2 cores.

Sharding: batch (4) x head-group (2x8 heads) -> 8 cores. Each core computes,
for one batch and 8 heads: QKV projections, causal softmax attention, and its
partial output projection. Host sums the two head-group partials per batch and
adds the output bias.

Device layout (per core, all matmul operands bf16, fp32 accumulation):
  qT/kT: [512, 2048] (head-major transposed projections), stored as 4
         "pair" tiles [128, 2048] (two 64-dim heads per tile) so that
         scoresT = kT.T @ qT runs as row-tiled K=64 matmul pairs.
  v:     [2048, 512] natural, tiles [128(s), 512(o)].
  scoresT blocks [128(k), 512(q)] in PSUM -> exp on ScalarE -> probsT bf16.
  out.T accumulated per head pair in PSUM via col-tiled M=64 matmul pairs;
  softmax denominators via ones-vector matmuls (M=1) at col offsets 0/32.
  Normalization: exact DVE reciprocal of the denominator rows, expanded
  across partitions with gpsimd partition_broadcast (+ one cross-partition
  copy), then one tensor_tensor multiply per unit.
  Final projection y = out @ WoT accumulates over the 4 pairs.

  Measured on 8 axon TRN2 cores: ~640 us HW exec, rel err ~0.0059 vs the
  fp32 reference (bf16 matmul precision).
"""

import sys

sys.path.insert(0, "/opt/trn_rl_repo")

import math
from contextlib import ExitStack

import numpy as np
import ml_dtypes

import concourse.bass as bass
import concourse.mybir as mybir
from concourse import bacc
from concourse.tile import TileContext
from concourse.tile_rust import add_dep_helper
from concourse.bass_utils import run_bass_kernel_spmd

BF16 = mybir.dt.bfloat16
F32 = mybir.dt.float32
AF = mybir.ActivationFunctionType
ALU = mybir.AluOpType

B, S, H = 4, 2048, 1024
NH, DH = 16, 64
O = 512          # per-core output dim of q/k/v projections (8 heads x 64)
NPAIR = 4        # head pairs per core
NSLAB = 4        # q slabs of 512
NST = 16         # s-tiles of 128
MASK_FILL = -8.0e5  # pre-scale (x0.125) additive mask for padded keys

_BUILT = {}


def _build(general_mask: bool):
    if general_mask in _BUILT:
        return _BUILT[general_mask]

    nc = bacc.Bacc("TRN2", target_bir_lowering=False, debug=False)

    xqT = nc.dram_tensor("xqT", [H, S], BF16, kind="ExternalInput")
    xkT = nc.dram_tensor("xkT", [H, S], BF16, kind="ExternalInput")
    xvT = nc.dram_tensor("xvT", [H, S], BF16, kind="ExternalInput")
    wqT = nc.dram_tensor("wqT", [H, O], BF16, kind="ExternalInput")
    wkT = nc.dram_tensor("wkT", [H, O], BF16, kind="ExternalInput")
    wvT = nc.dram_tensor("wvT", [H, O], BF16, kind="ExternalInput")
    woT = nc.dram_tensor("woT", [O, H], BF16, kind="ExternalInput")
    bqc = nc.dram_tensor("bqc", [128, 4], F32, kind="ExternalInput")
    bkc = nc.dram_tensor("bkc", [128, 4], F32, kind="ExternalInput")
    bvr = nc.dram_tensor("bvr", [1, O], BF16, kind="ExternalInput")
    tri = nc.dram_tensor("tri", [128, 896], BF16, kind="ExternalInput")
    mb = nc.dram_tensor("mb", [1, S], BF16, kind="ExternalInput")
    y = nc.dram_tensor("y", [S, H], F32, kind="ExternalOutput")

    with TileContext(nc) as tc, ExitStack() as ctx:
        P = lambda name, bufs, **kw: ctx.enter_context(
            tc.tile_pool(name=name, bufs=bufs, **kw)
        )
        wp = P("wp", 1)
        xt = P("xt", 2)
        qk = P("qk", 1)
        vp = P("vp", 1)
        pb = P("pb", 6)                    # probsT bf16 groups
        ev = P("ev", 3)                    # evacuation temps
        ot = P("ot", 8)                    # outT_sb bf16, held per slab
        ys = P("ys", 4)                    # y sbuf staging
        dp = P("dp", 4, space="DRAM")      # denominators round-trip

        # --- constants / weights ---
        wq_sb = wp.tile([128, 8, O], BF16, tag="wq", name="wq")
        wk_sb = wp.tile([128, 8, O], BF16, tag="wk", name="wk")
        wv_sb = wp.tile([128, 8, O], BF16, tag="wv", name="wv")
        wo_sb = wp.tile([128, 4, H], BF16, tag="wo", name="wo")
        nc.sync.dma_start(wq_sb[:], wqT.rearrange("(po pi) o -> pi po o", pi=128))
        nc.sync.dma_start(wk_sb[:], wkT.rearrange("(po pi) o -> pi po o", pi=128))
        nc.sync.dma_start(wv_sb[:], wvT.rearrange("(po pi) o -> pi po o", pi=128))
        nc.sync.dma_start(wo_sb[:], woT.rearrange("(po pi) j -> pi po j", pi=128))
        bq_sb = wp.tile([128, 4], F32, tag="bq", name="bq")
        bk_sb = wp.tile([128, 4], F32, tag="bk", name="bk")
        bv_sb = wp.tile([1, O], BF16, tag="bv", name="bv")
        tri_sb = wp.tile([128, 896], BF16, tag="tri", name="tri")
        nc.sync.dma_start(bq_sb[:], bqc[:, :])
        nc.sync.dma_start(bk_sb[:], bkc[:, :])
        nc.sync.dma_start(bv_sb[:], bvr[:, :])
        nc.sync.dma_start(tri_sb[:], tri[:, :])
        ones_col = wp.tile([128, 1], BF16, tag="onc", name="onc")
        zeros_col = wp.tile([128, 1], F32, tag="zc", name="zc")
        nc.gpsimd.memset(zeros_col[:], 0.0)
        ones_row = wp.tile([1, 512], BF16, tag="onr", name="onr")
        nc.gpsimd.memset(ones_col[:], 1.0)
        nc.gpsimd.memset(ones_row[:], 1.0)
        if general_mask:
            mb_sb = wp.tile([1, S], BF16, tag="mb", name="mb")
            nc.sync.dma_start(mb_sb[:], mb[:, :])

        # --- projections ---
        qT_sb = [qk.tile([128, S], BF16, tag=f"qT{p}", name=f"qT{p}") for p in range(NPAIR)]
        kT_sb = [qk.tile([128, S], BF16, tag=f"kT{p}", name=f"kT{p}") for p in range(NPAIR)]
        v_sb = vp.tile([128, NST, O], BF16, tag="v", name="v")

        def load_xt(dram):
            t = xt.tile([128, 8, S], BF16, tag="xt", name="xt")
            nc.sync.dma_start(t[:], dram.rearrange("(po pi) s -> pi po s", pi=128))
            return t

        xq_t = load_xt(xqT)
        xk_t = load_xt(xkT)

        def project_qk(x_t, w_sb, b_sb, dst, pp):
            # dst[m][o_in_pair, s] = sum_i w[i, 128m + o] x[i, s] + b
            for m in range(4):
                for sl in range(4):
                    ps = pp.tile([128, 512], F32, tag="pp", name="pp")
                    for ic in range(8):
                        nc.tensor.matmul(
                            ps[:],
                            w_sb[:, ic, 128 * m : 128 * m + 128],
                            x_t[:, ic, 512 * sl : 512 * sl + 512],
                            start=(ic == 0),
                            stop=(ic == 7),
                        )
                    nc.vector.tensor_scalar_add(
                        dst[m][:, 512 * sl : 512 * sl + 512], ps[:], b_sb[:, m : m + 1]
                    )

        with tc.tile_pool(name="pp", bufs=4, space="PSUM") as pp:
            project_qk(xq_t, wq_sb, bq_sb, qT_sb, pp)
            xv_t = load_xt(xvT)
            project_qk(xk_t, wk_sb, bk_sb, kT_sb, pp)

            # v natural: v[s, o] = sum_i x[i, s] w[i, o] + bv[o]
            for st in range(NST):
                ps = pp.tile([128, 512], F32, tag="pp", name="pp")
                for ic in range(8):
                    nc.tensor.matmul(
                        ps[:],
                        xv_t[:, ic, 128 * st : 128 * st + 128],
                        wv_sb[:, ic, :],
                        start=(ic == 0),
                        stop=False,
                    )
                nc.tensor.matmul(
                    ps[:], ones_row[:, 0:128], bv_sb[:, :], start=False, stop=True
                )
                nc.vector.tensor_copy(v_sb[:, st, :], ps[:])

        scp = P("scp", 1, space="PSUM")    # scoresT groups [128,2048]
        otp = P("otp", 2, space="PSUM")    # outT [128,512]
        lp = P("lp", 1, space="PSUM")      # denominators [64,512]
        yp = P("yp", 1, space="PSUM")      # final y [128,512]

        # --- attention + final projection, slab by slab ---
        for slab in range(NSLAB):
            out_sb_tiles = []
            for pair in range(NPAIR):
                n_kt = 4 * (slab + 1)
                q0 = 512 * slab
                ot_ps = otp.tile([128, 512], F32, tag="ot", name="ot")
                l_ps = lp.tile([64, 512], F32, tag="l", name="l")

                def chained_mm(bank, out_ap, lhsT, rhs, start, stop):
                    # HW-verified: start=True clears has_written only for the
                    # written partition slice, so the two col-tiled heads can
                    # run independent accumulation groups in one bank. The
                    # sim's global group-check mis-models partition-offset
                    # outputs, hence skip_group_check.
                    nc.tensor.matmul(
                        out_ap, lhsT, rhs, start=start, stop=stop,
                        skip_group_check=True,
                    )
                for g in range((n_kt + 1) // 2):
                    kts = [kt for kt in (2 * g, 2 * g + 1) if kt < n_kt]
                    used = 512 * len(kts)
                    scA = scp.tile([128, 1024], F32, tag="sc", name="sc")
                    scB = scp.tile([128, 1024], F32, tag="sc", name="sc")
                    for j, kt in enumerate(kts):
                        for hh, sc in ((0, scA), (1, scB)):
                            r0 = 64 * hh
                            nc.tensor.matmul(
                                sc[:, 512 * j : 512 * j + 512],
                                kT_sb[pair][r0 : r0 + 64, 128 * kt : 128 * kt + 128],
                                qT_sb[pair][r0 : r0 + 64, q0 : q0 + 512],
                                start=True,
                                stop=not general_mask,
                            )
                            if general_mask:
                                nc.tensor.matmul(
                                    sc[:, 512 * j : 512 * j + 512],
                                    mb_sb[0:1, 128 * kt : 128 * kt + 128],
                                    ones_row[0:1, :],
                                    start=False,
                                    stop=True,
                                )
                    pbA = pb.tile([128, 1024], BF16, tag="pb", name="pb")
                    pbB = pb.tile([128, 1024], BF16, tag="pb", name="pb")
                    nc.scalar.activation(
                        pbA[:, 0:used], scA[:, 0:used], AF.Exp,
                        bias=zeros_col[:, 0:1], scale=0.125,
                    )
                    nc.scalar.activation(
                        pbB[:, 0:used], scB[:, 0:used], AF.Exp,
                        bias=zeros_col[:, 0:1], scale=0.125,
                    )
                    for j, kt in enumerate(kts):
                        js = slice(512 * j, 512 * j + 512)
                        if kt >= 4 * slab:  # diagonal block: causal triangle
                            o = 128 * (kt - 4 * slab)
                            w = o + 128  # cols >= o+128 are all-ones: skip
                            for p_t in (pbA, pbB):
                                nc.vector.tensor_tensor(
                                    p_t[:, 512 * j : 512 * j + w],
                                    p_t[:, 512 * j : 512 * j + w],
                                    tri_sb[:, 384 - o : 384 - o + w],
                                    ALU.mult,
                                )
                        for hh, p_t in ((0, pbA), (1, pbB)):
                            # one accumulation group per PSUM bank: only the
                            # very first matmul may use start=True (it clears
                            # the whole bank's has_written bits)
                            st = kt == 0
                            sp = kt == n_kt - 1
                            chained_mm(
                                "ot",
                                ot_ps[64 * hh : 64 * hh + 64, :],
                                v_sb[:, kt, 64 * (2 * pair + hh) : 64 * (2 * pair + hh) + 64],
                                p_t[:, js],
                                st,
                                sp,
                            )
                            lr0 = 32 * hh
                            chained_mm(
                                "l",
                                l_ps[lr0 : lr0 + 1, :],
                                ones_col[:, :],
                                p_t[:, js],
                                st,
                                sp,
                            )
                # normalization: recip rows -> DRAM -> broadcast back
                lsb = ev.tile([33, 512], F32, tag="lsb", name="lsb")
                nc.gpsimd.memset(lsb[:], 1.0)
                nc.vector.tensor_copy(lsb[0:1, :], l_ps[0:1, :])
                nc.vector.tensor_copy(lsb[32:33, :], l_ps[32:33, :])
                lrec = ev.tile([33, 512], F32, tag="lrec", name="lrec")
                nc.vector.reciprocal(lrec[:], lsb[:])
                lra = lrec
                lrb = ev.tile([1, 512], F32, tag="lrb", name="lrb")
                nc.vector.tensor_copy(lrb[0:1, :], lrec[32:33, :])
                rx = ev.tile([128, 512], F32, tag="rx", name="rx")
                rxb = ev.tile([128, 512], F32, tag="rxb", name="rxb")
                nc.gpsimd.partition_broadcast(rx[0:128, :], lra[0:1, :])
                nc.gpsimd.partition_broadcast(rxb[0:128, :], lrb[0:1, :])
                nc.vector.tensor_copy(rx[64:128, :], rxb[64:128, :])
                o_sb = ot.tile([128, 512], BF16, tag="ot", name="ot")
                nc.vector.tensor_tensor(o_sb[:], ot_ps[:], rx[:], ALU.mult)
                out_sb_tiles.append(o_sb)

            # final projection for this slab of queries
            for st in range(4):
                srow = 512 * slab + 128 * st
                for jsl in range(2):
                    y_ps = yp.tile([128, 512], F32, tag="yp", name="yp")
                    for pair in range(NPAIR):
                        nc.tensor.matmul(
                            y_ps[:],
                            out_sb_tiles[pair][:, 128 * st : 128 * st + 128],
                            wo_sb[:, pair, 512 * jsl : 512 * jsl + 512],
                            start=(pair == 0),
                            stop=(pair == NPAIR - 1),
                        )
                    ysb = ys.tile([128, 512], F32, tag="ys", name="ys")
                    nc.vector.tensor_copy(ysb[:], y_ps[:])
                    nc.sync.dma_start(
                        y[srow : srow + 128, 512 * jsl : 512 * jsl + 512], ysb[:]
                    )

    nc.compile()
    _BUILT[general_mask] = nc
    return nc


def _prep_core(query, key, value, mask, Wq, bq, Wk, bk, Wv, bv, Wo, core):
    b, hg = core // 2, core % 2
    o_sl = slice(hg * O, hg * O + O)
    bf = ml_dtypes.bfloat16

    tri = np.zeros((128, 896), dtype=np.float32)
    j = np.arange(896)[None, :]
    kk = np.arange(128)[:, None]
    tri[(j - 384) >= kk] = 1.0

    mrow = np.where(mask[b] > 0, 0.0, MASK_FILL).astype(np.float32)

    return {
        "xqT": np.ascontiguousarray(query[b].T).astype(bf),
        "xkT": np.ascontiguousarray(key[b].T).astype(bf),
        "xvT": np.ascontiguousarray(value[b].T).astype(bf),
        "wqT": np.ascontiguousarray(Wq[o_sl].T).astype(bf),
        "wkT": np.ascontiguousarray(Wk[o_sl].T).astype(bf),
        "wvT": np.ascontiguousarray(Wv[o_sl].T).astype(bf),
        "woT": np.ascontiguousarray(Wo[:, o_sl].T).astype(bf),
        "bqc": np.ascontiguousarray(bq[o_sl].reshape(4, 128).T).astype(np.float32),
        "bkc": np.ascontiguousarray(bk[o_sl].reshape(4, 128).T).astype(np.float32),
        "bvr": bv[o_sl].reshape(1, O).astype(bf),
        "tri": tri.astype(bf),
        "mb": mrow.reshape(1, S).astype(bf),
    }


def kernel(query, key, value, mask, Wq, bq, Wk, bk, Wv, bv, Wo, bo, _trace=False):
    general_mask = bool(np.any(np.asarray(mask) <= 0))
    nc = _build(general_mask)
    in_maps = [
        _prep_core(query, key, value, mask, Wq, bq, Wk, bk, Wv, bv, Wo, c)
        for c in range(8)
    ]
    res = run_bass_kernel_spmd(
        nc, in_maps, core_ids=list(range(8)), trace=_trace,
        trace_cores=list(range(8)) if _trace else None,
    )
    parts = np.stack([res.results[c]["y"] for c in range(8)])  # [8, S, H]
    out = parts[0::2] + parts[1::2] + np.asarray(bo)[None, None, :]
    if _trace:
        kernel.last_results = res
    return out.astype(np.float32)


# revision 31
# speedup vs baseline: 1.4688x; 1.4688x over previous
"""Multi-head causal attention (B=4, S=2048, H=1024, 16 heads) on 8 TRN2 cores.

Sharding: batch (4) x head-group (2x8 heads) -> 8 cores. Each core computes,
for one batch and 8 heads: QKV projections, causal softmax attention, and its
partial output projection. Host sums the two head-group partials per batch and
adds the output bias.

Device layout (per core, all matmul operands bf16, fp32 accumulation):
  qT/kT: [512, 2048] (head-major transposed projections), stored as 4
         "pair" tiles [128, 2048] (two 64-dim heads per tile) so that
         scoresT = kT.T @ qT runs as row-tiled K=64 matmul pairs.
  v:     [2048, 512] natural, tiles [128(s), 512(o)].
  scoresT blocks [128(k), 512(q)] in PSUM -> exp on ScalarE -> probsT bf16.
  out.T accumulated per head pair in PSUM via col-tiled M=64 matmul pairs;
  softmax denominators via ones-vector matmuls (M=1) at col offsets 0/32.
  Normalization: exact DVE reciprocal of the denominator rows, expanded
  across partitions with gpsimd partition_broadcast (+ one cross-partition
  copy), then one tensor_tensor multiply per unit.
  Final projection y = out @ WoT accumulates over the 4 pairs.

  Measured on 8 axon TRN2 cores: ~640 us HW exec, rel err ~0.0059 vs the
  fp32 reference (bf16 matmul precision).
"""

import sys

sys.path.insert(0, "/opt/trn_rl_repo")

import math
from contextlib import ExitStack

import numpy as np
import ml_dtypes

import concourse.bass as bass
import concourse.mybir as mybir
from concourse import bacc
from concourse.tile import TileContext
from concourse.tile_rust import add_dep_helper
from concourse.bass_utils import run_bass_kernel_spmd

BF16 = mybir.dt.bfloat16
F32 = mybir.dt.float32
AF = mybir.ActivationFunctionType
ALU = mybir.AluOpType

B, S, H = 4, 2048, 1024
NH, DH = 16, 64
O = 512          # per-core output dim of q/k/v projections (8 heads x 64)
NPAIR = 4        # head pairs per core
NSLAB = 4        # q slabs of 512
NST = 16         # s-tiles of 128
MASK_FILL = -8.0e5  # pre-scale (x0.125) additive mask for padded keys

_BUILT = {}


def _build(general_mask: bool):
    if general_mask in _BUILT:
        return _BUILT[general_mask]

    nc = bacc.Bacc("TRN2", target_bir_lowering=False, debug=False)

    xqT = nc.dram_tensor("xqT", [H, S], BF16, kind="ExternalInput")
    xkT = nc.dram_tensor("xkT", [H, S], BF16, kind="ExternalInput")
    xvT = nc.dram_tensor("xvT", [H, S], BF16, kind="ExternalInput")
    wqT = nc.dram_tensor("wqT", [H, O], BF16, kind="ExternalInput")
    wkT = nc.dram_tensor("wkT", [H, O], BF16, kind="ExternalInput")
    wvT = nc.dram_tensor("wvT", [H, O], BF16, kind="ExternalInput")
    woT = nc.dram_tensor("woT", [O, H], BF16, kind="ExternalInput")
    bqc = nc.dram_tensor("bqc", [128, 4], F32, kind="ExternalInput")
    bkc = nc.dram_tensor("bkc", [128, 4], F32, kind="ExternalInput")
    bvr = nc.dram_tensor("bvr", [1, O], BF16, kind="ExternalInput")
    tri = nc.dram_tensor("tri", [128, 896], BF16, kind="ExternalInput")
    mb = nc.dram_tensor("mb", [1, S], BF16, kind="ExternalInput")
    y = nc.dram_tensor("y", [S, H], F32, kind="ExternalOutput")

    with TileContext(nc) as tc, ExitStack() as ctx:
        P = lambda name, bufs, **kw: ctx.enter_context(
            tc.tile_pool(name=name, bufs=bufs, **kw)
        )
        wp = P("wp", 1)
        xt = P("xt", 2)
        qk = P("qk", 1)
        vp = P("vp", 1)
        pb = P("pb", 6)                    # probsT bf16 groups
        ev = P("ev", 3)                    # evacuation temps
        ot = P("ot", 8)                    # outT_sb bf16, held per slab
        ys = P("ys", 4)                    # y sbuf staging
        dp = P("dp", 4, space="DRAM")      # denominators round-trip

        # --- constants / weights ---
        wq_sb = wp.tile([128, 8, O], BF16, tag="wq", name="wq")
        wk_sb = wp.tile([128, 8, O], BF16, tag="wk", name="wk")
        wv_sb = wp.tile([128, 8, O], BF16, tag="wv", name="wv")
        wo_sb = wp.tile([128, 4, H], BF16, tag="wo", name="wo")
        nc.sync.dma_start(wq_sb[:], wqT.rearrange("(po pi) o -> pi po o", pi=128))
        nc.sync.dma_start(wk_sb[:], wkT.rearrange("(po pi) o -> pi po o", pi=128))
        nc.sync.dma_start(wv_sb[:], wvT.rearrange("(po pi) o -> pi po o", pi=128))
        nc.sync.dma_start(wo_sb[:], woT.rearrange("(po pi) j -> pi po j", pi=128))
        bq_sb = wp.tile([128, 4], F32, tag="bq", name="bq")
        bk_sb = wp.tile([128, 4], F32, tag="bk", name="bk")
        bv_sb = wp.tile([1, O], BF16, tag="bv", name="bv")
        tri_sb = wp.tile([128, 896], BF16, tag="tri", name="tri")
        nc.sync.dma_start(bq_sb[:], bqc[:, :])
        nc.sync.dma_start(bk_sb[:], bkc[:, :])
        nc.sync.dma_start(bv_sb[:], bvr[:, :])
        nc.sync.dma_start(tri_sb[:], tri[:, :])
        ones_col = wp.tile([128, 1], BF16, tag="onc", name="onc")
        zeros_col = wp.tile([128, 1], F32, tag="zc", name="zc")
        nc.gpsimd.memset(zeros_col[:], 0.0)
        ones_row = wp.tile([1, 512], BF16, tag="onr", name="onr")
        nc.gpsimd.memset(ones_col[:], 1.0)
        nc.gpsimd.memset(ones_row[:], 1.0)
        if general_mask:
            mb_sb = wp.tile([1, S], BF16, tag="mb", name="mb")
            nc.sync.dma_start(mb_sb[:], mb[:, :])

        # --- projections ---
        qT_sb = [qk.tile([128, S], BF16, tag=f"qT{p}", name=f"qT{p}") for p in range(NPAIR)]
        kT_sb = [qk.tile([128, S], BF16, tag=f"kT{p}", name=f"kT{p}") for p in range(NPAIR)]
        v_sb = vp.tile([128, NST, O], BF16, tag="v", name="v")

        def load_xt(dram):
            t = xt.tile([128, 8, S], BF16, tag="xt", name="xt")
            nc.sync.dma_start(t[:], dram.rearrange("(po pi) s -> pi po s", pi=128))
            return t

        xq_t = load_xt(xqT)
        xk_t = load_xt(xkT)

        def project_qk(x_t, w_sb, b_sb, dst, pp):
            # dst[m][o_in_pair, s] = sum_i w[i, 128m + o] x[i, s] + b
            for m in range(4):
                for sl in range(4):
                    ps = pp.tile([128, 512], F32, tag="pp", name="pp")
                    for ic in range(8):
                        nc.tensor.matmul(
                            ps[:],
                            w_sb[:, ic, 128 * m : 128 * m + 128],
                            x_t[:, ic, 512 * sl : 512 * sl + 512],
                            start=(ic == 0),
                            stop=(ic == 7),
                        )
                    nc.vector.tensor_scalar_add(
                        dst[m][:, 512 * sl : 512 * sl + 512], ps[:], b_sb[:, m : m + 1]
                    )

        with tc.tile_pool(name="pp", bufs=4, space="PSUM") as pp:
            project_qk(xq_t, wq_sb, bq_sb, qT_sb, pp)
            xv_t = load_xt(xvT)
            project_qk(xk_t, wk_sb, bk_sb, kT_sb, pp)

            # v natural: v[s, o] = sum_i x[i, s] w[i, o] + bv[o]
            for st in range(NST):
                ps = pp.tile([128, 512], F32, tag="pp", name="pp")
                for ic in range(8):
                    nc.tensor.matmul(
                        ps[:],
                        xv_t[:, ic, 128 * st : 128 * st + 128],
                        wv_sb[:, ic, :],
                        start=(ic == 0),
                        stop=False,
                    )
                nc.tensor.matmul(
                    ps[:], ones_row[:, 0:128], bv_sb[:, :], start=False, stop=True
                )
                nc.vector.tensor_copy(v_sb[:, st, :], ps[:])

        scp = P("scp", 2, space="PSUM")    # scoresT groups [128,1024]
        otp = P("otp", 2, space="PSUM")    # outT [128,512]
        lp = P("lp", 1, space="PSUM")      # denominators [64,512]
        yp = P("yp", 1, space="PSUM")      # final y [128,512]

        # --- attention + final projection, slab by slab ---
        for slab in range(NSLAB):
            out_sb_tiles = []
            for pair in range(NPAIR):
                n_kt = 4 * (slab + 1)
                q0 = 512 * slab
                ot_ps = otp.tile([128, 512], F32, tag="ot", name="ot")
                l_ps = lp.tile([64, 512], F32, tag="l", name="l")

                def chained_mm(bank, out_ap, lhsT, rhs, start, stop):
                    # HW-verified: start=True clears has_written only for the
                    # written partition slice, so the two col-tiled heads can
                    # run independent accumulation groups in one bank. The
                    # sim's global group-check mis-models partition-offset
                    # outputs, hence skip_group_check.
                    nc.tensor.matmul(
                        out_ap, lhsT, rhs, start=start, stop=stop,
                        skip_group_check=True,
                    )
                for g in range((n_kt + 1) // 2):
                    kts = [kt for kt in (2 * g, 2 * g + 1) if kt < n_kt]
                    used = 512 * len(kts)
                    scA = scp.tile([128, 1024], F32, tag="sc", name="sc")
                    scB = scp.tile([128, 1024], F32, tag="sc", name="sc")
                    for j, kt in enumerate(kts):
                        for hh, sc in ((0, scA), (1, scB)):
                            r0 = 64 * hh
                            nc.tensor.matmul(
                                sc[:, 512 * j : 512 * j + 512],
                                kT_sb[pair][r0 : r0 + 64, 128 * kt : 128 * kt + 128],
                                qT_sb[pair][r0 : r0 + 64, q0 : q0 + 512],
                                start=True,
                                stop=not general_mask,
                            )
                            if general_mask:
                                nc.tensor.matmul(
                                    sc[:, 512 * j : 512 * j + 512],
                                    mb_sb[0:1, 128 * kt : 128 * kt + 128],
                                    ones_row[0:1, :],
                                    start=False,
                                    stop=True,
                                )
                    pbA = pb.tile([128, 1024], BF16, tag="pb", name="pb")
                    pbB = pb.tile([128, 1024], BF16, tag="pb", name="pb")
                    nc.scalar.activation(
                        pbA[:, 0:used], scA[:, 0:used], AF.Exp,
                        bias=zeros_col[:, 0:1], scale=0.125,
                    )
                    nc.scalar.activation(
                        pbB[:, 0:used], scB[:, 0:used], AF.Exp,
                        bias=zeros_col[:, 0:1], scale=0.125,
                    )
                    for j, kt in enumerate(kts):
                        js = slice(512 * j, 512 * j + 512)
                        if kt >= 4 * slab:  # diagonal block: causal triangle
                            o = 128 * (kt - 4 * slab)
                            w = o + 128  # cols >= o+128 are all-ones: skip
                            for p_t in (pbA, pbB):
                                nc.vector.tensor_tensor(
                                    p_t[:, 512 * j : 512 * j + w],
                                    p_t[:, 512 * j : 512 * j + w],
                                    tri_sb[:, 384 - o : 384 - o + w],
                                    ALU.mult,
                                )
                        for hh, p_t in ((0, pbA), (1, pbB)):
                            # one accumulation group per PSUM bank: only the
                            # very first matmul may use start=True (it clears
                            # the whole bank's has_written bits)
                            st = kt == 0
                            sp = kt == n_kt - 1
                            chained_mm(
                                "ot",
                                ot_ps[64 * hh : 64 * hh + 64, :],
                                v_sb[:, kt, 64 * (2 * pair + hh) : 64 * (2 * pair + hh) + 64],
                                p_t[:, js],
                                st,
                                sp,
                            )
                            lr0 = 32 * hh
                            chained_mm(
                                "l",
                                l_ps[lr0 : lr0 + 1, :],
                                ones_col[:, :],
                                p_t[:, js],
                                st,
                                sp,
                            )
                # normalization: recip rows -> DRAM -> broadcast back
                lsb = ev.tile([33, 512], F32, tag="lsb", name="lsb")
                nc.gpsimd.memset(lsb[:], 1.0)
                nc.vector.tensor_copy(lsb[0:1, :], l_ps[0:1, :])
                nc.vector.tensor_copy(lsb[32:33, :], l_ps[32:33, :])
                lrec = ev.tile([33, 512], F32, tag="lrec", name="lrec")
                nc.vector.reciprocal(lrec[:], lsb[:])
                lra = lrec
                lrb = ev.tile([1, 512], F32, tag="lrb", name="lrb")
                nc.vector.tensor_copy(lrb[0:1, :], lrec[32:33, :])
                rx = ev.tile([128, 512], F32, tag="rx", name="rx")
                rxb = ev.tile([128, 512], F32, tag="rxb", name="rxb")
                nc.gpsimd.partition_broadcast(rx[0:128, :], lra[0:1, :])
                nc.gpsimd.partition_broadcast(rxb[0:128, :], lrb[0:1, :])
                nc.vector.tensor_copy(rx[64:128, :], rxb[64:128, :])
                o_sb = ot.tile([128, 512], BF16, tag="ot", name="ot")
                nc.vector.tensor_tensor(o_sb[:], ot_ps[:], rx[:], ALU.mult)
                out_sb_tiles.append(o_sb)

            # final projection for this slab of queries
            for st in range(4):
                srow = 512 * slab + 128 * st
                for jsl in range(2):
                    y_ps = yp.tile([128, 512], F32, tag="yp", name="yp")
                    for pair in range(NPAIR):
                        nc.tensor.matmul(
                            y_ps[:],
                            out_sb_tiles[pair][:, 128 * st : 128 * st + 128],
                            wo_sb[:, pair, 512 * jsl : 512 * jsl + 512],
                            start=(pair == 0),
                            stop=(pair == NPAIR - 1),
                        )
                    ysb = ys.tile([128, 512], F32, tag="ys", name="ys")
                    nc.vector.tensor_copy(ysb[:], y_ps[:])
                    nc.sync.dma_start(
                        y[srow : srow + 128, 512 * jsl : 512 * jsl + 512], ysb[:]
                    )

    nc.compile()
    _BUILT[general_mask] = nc
    return nc


def _prep_core(query, key, value, mask, Wq, bq, Wk, bk, Wv, bv, Wo, core):
    b, hg = core // 2, core % 2
    o_sl = slice(hg * O, hg * O + O)
    bf = ml_dtypes.bfloat16

    tri = np.zeros((128, 896), dtype=np.float32)
    j = np.arange(896)[None, :]
    kk = np.arange(128)[:, None]
    tri[(j - 384) >= kk] = 1.0

    mrow = np.where(mask[b] > 0, 0.0, MASK_FILL).astype(np.float32)

    return {
        "xqT": np.ascontiguousarray(query[b].T).astype(bf),
        "xkT": np.ascontiguousarray(key[b].T).astype(bf),
        "xvT": np.ascontiguousarray(value[b].T).astype(bf),
        "wqT": np.ascontiguousarray(Wq[o_sl].T).astype(bf),
        "wkT": np.ascontiguousarray(Wk[o_sl].T).astype(bf),
        "wvT": np.ascontiguousarray(Wv[o_sl].T).astype(bf),
        "woT": np.ascontiguousarray(Wo[:, o_sl].T).astype(bf),
        "bqc": np.ascontiguousarray(bq[o_sl].reshape(4, 128).T).astype(np.float32),
        "bkc": np.ascontiguousarray(bk[o_sl].reshape(4, 128).T).astype(np.float32),
        "bvr": bv[o_sl].reshape(1, O).astype(bf),
        "tri": tri.astype(bf),
        "mb": mrow.reshape(1, S).astype(bf),
    }


def kernel(query, key, value, mask, Wq, bq, Wk, bk, Wv, bv, Wo, bo, _trace=False):
    general_mask = bool(np.any(np.asarray(mask) <= 0))
    nc = _build(general_mask)
    in_maps = [
        _prep_core(query, key, value, mask, Wq, bq, Wk, bk, Wv, bv, Wo, c)
        for c in range(8)
    ]
    res = run_bass_kernel_spmd(
        nc, in_maps, core_ids=list(range(8)), trace=_trace,
        trace_cores=list(range(8)) if _trace else None,
    )
    parts = np.stack([res.results[c]["y"] for c in range(8)])  # [8, S, H]
    out = parts[0::2] + parts[1::2] + np.asarray(bo)[None, None, :]
    if _trace:
        kernel.last_results = res
    return out.astype(np.float32)
